# revision 66
# baseline (speedup 1.0000x reference)
"""Radix-4 DIF ambiguity kernel (bf16, host-R upload, chunk-pipelined).

Host precomputes the lag products R[k,t] = s[t]*conj(s[t-k]) (<0.2% of total
FLOPs) and uploads them chunk-major in u-tile layout; the device does the
radix-4 FFT4 combine (DVE bf16 2x ops), 4-branch DFT-256 matmuls (PE bf16,
512-wide moving tables), |X|^2 via ACT squares + DVE pair-adds into
r-plane-layout bf16 output. Normalization is exact-by-construction
(Cauchy-Schwarz: max chi = (sum|s|^2)^2) and folded into a host prescale of
s. Only k in [0,512) is computed on device; row k=512, the r-plane
interleave, the f32 cast, and the mirror half-plane chi[k,f] = chi[N-k,-f]
are assembled host-side during unsharding. All input DMAs ride one HWDGE
ring, hand-ordered to match consumption; B-tables are synthesized on-device
from the A-tables (tB = [Ms|Mc] is a column-swap+negate of tA = [Mc|-Ms]);
k-columns are chunked so PE starts ~11us in and all engines pipeline across
the two batches per core.
"""

import numpy as np
import ml_dtypes

import bass_rust
import concourse.bass as bass
import concourse.mybir as mybir
import concourse.tile as tile
import concourse.bass_utils as bass_utils

B, N = 16, 1024
NCORES = 8
BPC = B // NCORES
K = 512

f32 = mybir.dt.float32
bf16 = mybir.dt.bfloat16
ALU = mybir.AluOpType

# (batch, lo, hi) k-column chunks; small head chunks for ramp
CHUNKS = [
    (0, 0, 128), (0, 128, 256), (0, 256, 512),
    (1, 0, 256), (1, 256, 512),
]

# per-chunk column offsets into the chunk-major rt layout
CH_OFF = []
_o = 0
for _b, _lo, _hi in CHUNKS:
    CH_OFF.append(_o)
    _o += 16 * (_hi - _lo)
assert _o == 2 * 16 * K


def _split_excess_waits(nc):
    for f in nc.m.functions:
        for blk in f.blocks:
            insts = list(blk.instructions)
            new_insts = []
            changed = False
            for inst in insts:
                si = inst.sync_info
                waits = list(si.on_wait) if (si is not None and si.on_wait) else []
                keep_n = 0 if isinstance(inst, mybir.InstDrain) else 1
                if len(waits) > keep_n:
                    changed = True
                    extra = waits[: len(waits) - keep_n]
                    keep = waits[len(waits) - keep_n:]
                    for w in extra:
                        nop = mybir.InstNoOp(
                            name=nc.get_next_instruction_name(), ins=[], outs=[]
                        )
                        nop.engine = inst.engine
                        nop.sync_info = bass_rust.SyncInfo(on_wait=[w], on_update=[])
                        new_insts.append(nop)
                    inst.sync_info = bass_rust.SyncInfo(
                        on_wait=keep,
                        on_update=list(si.on_update) if si.on_update else [],
                    )
                new_insts.append(inst)
            if changed:
                blk.instructions = new_insts
    return nc


def build_nc():
    nc = bass.Bass("TRN2", target_bir_lowering=False, debug=False)

    # chunk-major R layout: for each chunk ci, a [128, 16*C] block at CH_OFF[ci]
    # with per-partition layout [j(8), reim(2), c(C)]
    rt = nc.dram_tensor("rt", [128, 2 * 16 * K], bf16, kind="ExternalInput")
    # A-tables only, r-major: slot (2r+h) = tA(r,h) = [Mc | -Ms]
    tabs_d = nc.dram_tensor("tabs", [128, 8 * 512], bf16, kind="ExternalInput")
    out = nc.dram_tensor("out", [BPC, K, N], bf16, kind="ExternalOutput")

    with tile.TileContext(nc) as tc:
        with (
            tc.tile_pool(name="const", bufs=1) as constp,
            tc.tile_pool(name="tmp", bufs=3) as tmpp,
            tc.tile_pool(name="bb", bufs=3) as bbp,
            tc.tile_pool(name="sq", bufs=4) as sqp,
            tc.tile_pool(name="chi", bufs=3) as chip,
            tc.tile_pool(name="ps", bufs=8, space="PSUM") as psp,
        ):
            TA = [constp.tile([128, 1024], bf16, tag=f"tA{r}", name=f"tA{r}")
                  for r in range(4)]
            TB = [constp.tile([128, 1024], bf16, tag=f"tB{r}", name=f"tB{r}")
                  for r in range(4)]

            def TT(form, r, h):
                t = TA[r] if form == "A" else TB[r]
                return t[:, h * 512:(h + 1) * 512]

            # ---- input loads ----
            # Everything on the SP HWDGE ring: FIFO per ring, so the order
            # below IS the arrival order. B-tables are synthesized on device:
            # tB = [Ms | Mc] from tA = [Mc | -Ms].
            UT = [None] * len(CHUNKS)

            def load_u(ci):
                b, lo, hi = CHUNKS[ci]
                C = hi - lo
                U = constp.tile([128, 16 * C], bf16, tag=f"u{ci}", name=f"u{ci}")
                nc.sync.dma_start(
                    U[:],
                    bass.AP(rt, CH_OFF[ci], [[2 * 16 * K, 128], [1, 16 * C]]),
                )
                UT[ci] = U

            def load_ta(r):
                nc.sync.dma_start(TA[r][:], tabs_d[:, r * 1024:(r + 1) * 1024])

            def _hp(t, off):
                # AP over both h-halves of a [128,1024] table tile
                ap = t[:]
                return bass.AP(ap.tensor, ap.offset + off,
                               [ap.ap[0], [512, 2], [1, 256]])

            def emit_tb(r):
                # B-table synthesis: negate half on DVE, copy half on ACT,
                # both h-halves per op (fewer instructions -> shorter
                # teardown sem chains)
                nc.vector.tensor_scalar_mul(_hp(TB[r], 0), _hp(TA[r], 256), -1.0)
                nc.scalar.copy(_hp(TB[r], 256), _hp(TA[r], 0))

            load_u(0)
            load_ta(0)
            load_ta(1)
            load_u(1)
            load_ta(2)
            load_ta(3)
            load_u(2)
            load_u(3)
            load_u(4)
            emit_tb(0)
            emit_tb(1)

            def emit_fft4(ci, b0_first=False):
                b, lo, hi = CHUNKS[ci]
                C = hi - lo
                U = UT[ci]

                def u(j):
                    return U[:, j * 2 * C:(j + 1) * 2 * C]

                Bt = {}
                pqw = {}
                tt = nc.vector.tensor_tensor

                def prereq(h):
                    u0, u1, u2, u3 = u(h), u(2 + h), u(4 + h), u(6 + h)
                    P = tmpp.tile([128, 2 * C], bf16, tag=f"P{h}", name=f"P{h}_{ci}")
                    Q = tmpp.tile([128, 2 * C], bf16, tag=f"Q{h}", name=f"Q{h}_{ci}")
                    U2 = tmpp.tile([128, 2 * C], bf16, tag=f"U{h}", name=f"U{h}_{ci}")
                    W = tmpp.tile([128, 2 * C], bf16, tag=f"W{h}", name=f"W{h}_{ci}")
                    for r in range(4):
                        Bt[(r, h)] = bbp.tile(
                            [128, 2 * C], bf16, tag=f"b{r}{h}", name=f"b{r}{h}_{ci}"
                        )
                    tt(P[:], u0, u2, op=ALU.add)
                    tt(Q[:], u0, u2, op=ALU.subtract)
                    tt(U2[:], u1, u3, op=ALU.add)
                    # W = (V.im | -V.re), V = u1 - u3; B1 = Q+W, B3 = Q-W
                    tt(W[:, 0:C], u1[:, C:2 * C], u3[:, C:2 * C], op=ALU.subtract)
                    tt(W[:, C:2 * C], u3[:, 0:C], u1[:, 0:C], op=ALU.subtract)
                    pqw[h] = (P, Q, U2, W)

                def emit_b(r, h):
                    P, Q, U2, W = pqw[h]
                    if r == 0:
                        tt(Bt[(0, h)][:], P[:], U2[:], op=ALU.add)
                    elif r == 1:
                        tt(Bt[(1, h)][:], Q[:], W[:], op=ALU.add)
                    elif r == 2:
                        tt(Bt[(2, h)][:], P[:], U2[:], op=ALU.subtract)
                    else:
                        tt(Bt[(3, h)][:], Q[:], W[:], op=ALU.subtract)

                if b0_first:
                    prereq(0)
                    for r in range(4):
                        emit_b(r, 0)
                    emit_tb(2)
                    prereq(1)
                    for r in range(4):
                        emit_b(r, 1)
                    emit_tb(3)
                else:
                    # h-merged temps: U's j-blocks pair (h0|h1) adjacently
                    # (j=2c+h), so each FFT4 temp line is ONE full-width op;
                    # B-tiles stay per-(r,h) to keep PE-feeding granularity
                    uap = U[:]

                    def cp(c):  # c-pair block [j=2c | j=2c+1]
                        return U[:, c * 4 * C:(c + 1) * 4 * C]

                    def u3(base, off):
                        return bass.AP(uap.tensor, uap.offset + base + off,
                                       [uap.ap[0], [2 * C, 2], [1, C]])

                    P2 = tmpp.tile([128, 4 * C], bf16, tag="P2", name=f"P2_{ci}")
                    Q2 = tmpp.tile([128, 4 * C], bf16, tag="Q2", name=f"Q2_{ci}")
                    U22 = tmpp.tile([128, 4 * C], bf16, tag="U22", name=f"U22_{ci}")
                    W2 = tmpp.tile([128, 4 * C], bf16, tag="W2", name=f"W2_{ci}")
                    w2ap = W2[:]

                    def w3(off):
                        return bass.AP(w2ap.tensor, w2ap.offset + off,
                                       [w2ap.ap[0], [2 * C, 2], [1, C]])

                    tt(P2[:], cp(0), cp(2), op=ALU.add)
                    tt(Q2[:], cp(0), cp(2), op=ALU.subtract)
                    tt(U22[:], cp(1), cp(3), op=ALU.add)
                    # W_h = (V_h.im | -V_h.re), V_h = u_{2+h} - u_{6+h}
                    tt(w3(0), u3(4 * C, C), u3(12 * C, C), op=ALU.subtract)
                    tt(w3(C), u3(12 * C, 0), u3(4 * C, 0), op=ALU.subtract)
                    for h in range(2):
                        sl = slice(h * 2 * C, (h + 1) * 2 * C)
                        for r in range(4):
                            Bt[(r, h)] = bbp.tile(
                                [128, 2 * C], bf16, tag=f"b{r}{h}",
                                name=f"b{r}{h}_{ci}"
                            )
                        tt(Bt[(0, h)][:], P2[:, sl], U22[:, sl], op=ALU.add)
                        tt(Bt[(1, h)][:], Q2[:, sl], W2[:, sl], op=ALU.add)
                        tt(Bt[(2, h)][:], P2[:, sl], U22[:, sl], op=ALU.subtract)
                        tt(Bt[(3, h)][:], Q2[:, sl], W2[:, sl], op=ALU.subtract)
                return Bt

            def emit_kblock(ci, Bt, kb, split_store=False, bb=0):
                # kb is the global kblock index (k rows 128*kb..128*kb+128)
                b, lo, hi = CHUNKS[ci]
                C = hi - lo
                c0 = 128 * kb - lo
                chi_t = chip.tile([128, N], bf16, tag="chi", name=f"chi{ci}_{kb}")
                sqm = None
                if not split_store:
                    # one [128,2048] sq tile per kblock -> single merged
                    # pair-add (fewer DVE ops: shorter runtime AND teardown)
                    sqm = sqp.tile([128, 2048], bf16, tag="sqm", name=f"sqm{ci}{kb}")
                for r in range(4):
                    ps = psp.tile([128, 512], f32, tag="ps", name=f"ps{ci}{kb}{r}")
                    if ci == 0:
                        # h-major MM order: h1's B-tiles land ~1us after h0's
                        # at the pipeline head, so do both h0 MMs first
                        order = [("A", 0), ("B", 0), ("A", 1), ("B", 1)]
                    else:
                        order = [("A", 0), ("A", 1), ("B", 0), ("B", 1)]
                    for i, (form, h) in enumerate(order):
                        off = c0 if form == "A" else C + c0
                        st = Bt[(r, h)][:, off:off + 128]
                        nc.tensor.matmul(ps[:], st, TT(form, r, h),
                                         start=(i == 0), stop=(i == 3))
                    if split_store:
                        # final kblock: per-r squares/pair-adds so each
                        # quarter drains via the idle SP queue immediately
                        sq = sqp.tile([128, 512], bf16, tag="sq", name=f"sq{ci}{kb}{r}")
                        nc.scalar.square(sq[:], ps[:])
                        nc.vector.tensor_tensor(
                            chi_t[:, r * 256:(r + 1) * 256],
                            sq[:, 0:256], sq[:, 256:512], op=ALU.add,
                        )
                        nc.sync.dma_start(
                            out[bb, 128 * kb:128 * kb + 128, r * 256:(r + 1) * 256],
                            chi_t[:, r * 256:(r + 1) * 256],
                        )
                    else:
                        nc.scalar.square(sqm[:, r * 512:(r + 1) * 512], ps[:])
                if not split_store:
                    # chi[r*256+q] = sq[r*512+q] + sq[r*512+256+q]
                    sap = sqm[:]
                    nc.vector.tensor_tensor(
                        chi_t[:],
                        bass.AP(sap.tensor, sap.offset,
                                [sap.ap[0], [512, 4], [1, 256]]),
                        bass.AP(sap.tensor, sap.offset + 256,
                                [sap.ap[0], [512, 4], [1, 256]]),
                        op=ALU.add,
                    )
                return chi_t

            def emit_store(b, kb, chi_t):
                # SP queue: keeps store descriptor-generation out of ACT's
                # FIFO and its sems off ACT's teardown chain
                nc.sync.dma_start(out[b, 128 * kb:128 * kb + 128, :], chi_t[:])

            # ---- schedule ----
            last_ci = len(CHUNKS) - 1
            for ci, (b, lo, hi) in enumerate(CHUNKS):
                Bt = emit_fft4(ci, b0_first=(ci == 0))
                for kb in range(lo // 128, hi // 128):
                    final = (ci == last_ci and kb == hi // 128 - 1)
                    chi_t = emit_kblock(ci, Bt, kb, split_store=final, bb=b)
                    if not final:
                        emit_store(b, kb, chi_t)

    _split_excess_waits(nc)
    return nc


_NC_CACHE = {}


def _get_nc():
    if "nc" not in _NC_CACHE:
        _NC_CACHE["nc"] = build_nc()
    return _NC_CACHE["nc"]


def _get_tables():
    if "tabs" not in _NC_CACHE:
        mpp = np.arange(256, dtype=np.float64)[:, None]
        t = np.arange(256, dtype=np.float64)[None, :]
        t_sh = (t + 128) % 256
        blocks = []
        for r in range(4):
            ang = 2.0 * np.pi * ((mpp * (r + 4 * t_sh)) % 1024) / 1024
            Mc = np.cos(ang)
            Ms = np.sin(ang)
            for h in range(2):
                sl = slice(128 * h, 128 * h + 128)
                blocks.append(np.concatenate([Mc[sl], -Ms[sl]], axis=1))
        big = np.concatenate(blocks, axis=1)
        _NC_CACHE["tabs"] = big.astype(ml_dtypes.bfloat16)
    return _NC_CACHE["tabs"]


def _host_prep(sr, si):
    """Per-core input prep. sr/si: [BPC, N] float32 (already prescaled).

    Chunk-major rt: for chunk ci=(b,lo,hi), block [128, 16*C] at CH_OFF[ci]
    where rt[p, off + j*2C + reim*C + c] = {re,im} R[k=lo+c, t=128j+p]."""
    Rts = []
    for b in range(BPC):
        s = sr[b].astype(np.complex64)
        s.imag = si[b]
        cs = np.conj(s)
        arr = np.concatenate([cs, cs])
        Wm = np.lib.stride_tricks.as_strided(
            arr[N:], shape=(N, K), strides=(arr.itemsize, -arr.itemsize))
        R = s[:, None] * Wm  # [t, k]
        Rb = np.empty((N, 2, K), dtype=np.float32)
        Rb[:, 0, :] = R.real
        Rb[:, 1, :] = R.imag
        Rts.append(Rb)
    rt = np.empty((128, 2 * 16 * K), dtype=ml_dtypes.bfloat16)
    for ci, (b, lo, hi) in enumerate(CHUNKS):
        C = hi - lo
        blk = Rts[b][:, :, lo:hi]                        # [t, 2, C]
        blk = blk.reshape(8, 128, 2, C).transpose(1, 0, 2, 3)  # [p, j, 2, C]
        rt[:, CH_OFF[ci]:CH_OFF[ci] + 16 * C] = blk.reshape(128, 16 * C).astype(
            ml_dtypes.bfloat16)
    return {"rt": rt, "tabs": _get_tables()}


def kernel(s_real: np.ndarray, s_imag: np.ndarray) -> np.ndarray:
    s_real = np.asarray(s_real, dtype=np.float32)
    s_imag = np.asarray(s_imag, dtype=np.float32)
    # exact normalization: max chi = (sum |s|^2)^2 (Cauchy-Schwarz, attained
    # at k=0,f=0), so prescale s by (sum|s|^2)^{-1/2}
    pw = (s_real.astype(np.float64) ** 2 + s_imag.astype(np.float64) ** 2).sum(
        axis=1, keepdims=True
    )
    g = 1.0 / np.sqrt(pw)
    sr_s = (s_real * g).astype(np.float32)
    si_s = (s_imag * g).astype(np.float32)

    nc = _get_nc()
    in_maps = [
        _host_prep(sr_s[c * BPC:(c + 1) * BPC], si_s[c * BPC:(c + 1) * BPC])
        for c in range(NCORES)
    ]
    res = bass_utils.run_bass_kernel_spmd(nc, in_maps, core_ids=list(range(NCORES)))
    planes = np.concatenate([r["out"] for r in res.results], axis=0)  # [B,512,N] bf16
    # r-plane interleave: chi[k, 4q+r] = planes[k, r*256+q]
    chi = (
        planes.astype(np.float32)
        .reshape(B, K, 4, 256)
        .transpose(0, 1, 3, 2)
        .reshape(B, K, N)
    )

    full = np.empty((B, N, N), dtype=np.float32)
    full[:, 512:1024, :] = chi
    # mirror: rows r in [1,512): chi[r] = flip_f(chi_direct[512 - r])
    src = chi[:, 511:0:-1, :]
    full[:, 1:512, 0] = src[:, :, 0]
    full[:, 1:512, 1:] = src[:, :, :0:-1]
    # row 0 (k=512) on host in float64
    s64 = (sr_s.astype(np.float64) + 1j * si_s.astype(np.float64))
    r512 = s64 * np.conj(np.roll(s64, 512, axis=1))
    x512 = np.fft.fft(r512, axis=1)
    full[:, 0, :] = np.fft.fftshift(
        (x512 * np.conj(x512)).real, axes=-1
    ).astype(np.float32)
    return full


# revision 67
# speedup vs baseline: 1.0304x; 1.0304x over previous
"""Radix-4 DIF ambiguity kernel (bf16, host-R upload, chunk-pipelined).

Host precomputes the lag products R[k,t] = s[t]*conj(s[t-k]) (<0.2% of total
FLOPs) and uploads them chunk-major in u-tile layout; the device does the
radix-4 FFT4 combine (DVE bf16 2x ops), 4-branch DFT-256 matmuls (PE bf16,
512-wide moving tables), |X|^2 via ACT squares + DVE pair-adds into
r-plane-layout bf16 output. Normalization is exact-by-construction
(Cauchy-Schwarz: max chi = (sum|s|^2)^2) and folded into a host prescale of
s. Only k in [0,512) is computed on device; row k=512, the r-plane
interleave, the f32 cast, and the mirror half-plane chi[k,f] = chi[N-k,-f]
are assembled host-side during unsharding. All input DMAs ride one HWDGE
ring, hand-ordered to match consumption; B-tables are synthesized on-device
from the A-tables (tB = [Ms|Mc] is a column-swap+negate of tA = [Mc|-Ms]);
k-columns are chunked so PE starts ~11us in and all engines pipeline across
the two batches per core.
"""

import numpy as np
import ml_dtypes

import bass_rust
import concourse.bass as bass
import concourse.mybir as mybir
import concourse.tile as tile
import concourse.bass_utils as bass_utils

B, N = 16, 1024
NCORES = 8
BPC = B // NCORES
K = 512

f32 = mybir.dt.float32
bf16 = mybir.dt.bfloat16
ALU = mybir.AluOpType

# (batch, lo, hi) k-column chunks; small head chunks for ramp
CHUNKS = [
    (0, 0, 128), (0, 128, 256), (0, 256, 512),
    (1, 0, 256), (1, 256, 512),
]

# per-chunk column offsets into the chunk-major rt layout
CH_OFF = []
_o = 0
for _b, _lo, _hi in CHUNKS:
    CH_OFF.append(_o)
    _o += 16 * (_hi - _lo)
assert _o == 2 * 16 * K


def _split_excess_waits(nc):
    for f in nc.m.functions:
        for blk in f.blocks:
            insts = list(blk.instructions)
            new_insts = []
            changed = False
            for inst in insts:
                si = inst.sync_info
                waits = list(si.on_wait) if (si is not None and si.on_wait) else []
                keep_n = 0 if isinstance(inst, mybir.InstDrain) else 1
                if len(waits) > keep_n:
                    changed = True
                    extra = waits[: len(waits) - keep_n]
                    keep = waits[len(waits) - keep_n:]
                    for w in extra:
                        nop = mybir.InstNoOp(
                            name=nc.get_next_instruction_name(), ins=[], outs=[]
                        )
                        nop.engine = inst.engine
                        nop.sync_info = bass_rust.SyncInfo(on_wait=[w], on_update=[])
                        new_insts.append(nop)
                    inst.sync_info = bass_rust.SyncInfo(
                        on_wait=keep,
                        on_update=list(si.on_update) if si.on_update else [],
                    )
                new_insts.append(inst)
            if changed:
                blk.instructions = new_insts
    return nc


def build_nc():
    nc = bass.Bass("TRN2", target_bir_lowering=False, debug=False)

    # chunk-major R layout: for each chunk ci, a [128, 16*C] block at CH_OFF[ci]
    # with per-partition layout [j(8), reim(2), c(C)]
    rt = nc.dram_tensor("rt", [128, 2 * 16 * K], bf16, kind="ExternalInput")
    # A-tables only, r-major: slot (2r+h) = tA(r,h) = [Mc | -Ms]
    tabs_d = nc.dram_tensor("tabs", [128, 8 * 512], bf16, kind="ExternalInput")
    out = nc.dram_tensor("out", [BPC, K, N], bf16, kind="ExternalOutput")

    with tile.TileContext(nc) as tc:
        with (
            tc.tile_pool(name="const", bufs=1) as constp,
            tc.tile_pool(name="tmp", bufs=3) as tmpp,
            tc.tile_pool(name="bb", bufs=3) as bbp,
            tc.tile_pool(name="sq", bufs=4) as sqp,
            tc.tile_pool(name="chi", bufs=3) as chip,
            tc.tile_pool(name="ps", bufs=8, space="PSUM") as psp,
        ):
            TA = [constp.tile([128, 1024], bf16, tag=f"tA{r}", name=f"tA{r}")
                  for r in range(4)]
            TB = [constp.tile([128, 1024], bf16, tag=f"tB{r}", name=f"tB{r}")
                  for r in range(4)]

            def TT(form, r, h):
                t = TA[r] if form == "A" else TB[r]
                return t[:, h * 512:(h + 1) * 512]

            # ---- input loads ----
            # Everything on the SP HWDGE ring: FIFO per ring, so the order
            # below IS the arrival order. B-tables are synthesized on device:
            # tB = [Ms | Mc] from tA = [Mc | -Ms].
            UT = [None] * len(CHUNKS)

            def load_u(ci):
                b, lo, hi = CHUNKS[ci]
                C = hi - lo
                U = constp.tile([128, 16 * C], bf16, tag=f"u{ci}", name=f"u{ci}")
                nc.sync.dma_start(
                    U[:],
                    bass.AP(rt, CH_OFF[ci], [[2 * 16 * K, 128], [1, 16 * C]]),
                )
                UT[ci] = U

            def load_ta(r):
                nc.sync.dma_start(TA[r][:], tabs_d[:, r * 1024:(r + 1) * 1024])

            def _hp(t, off):
                # AP over both h-halves of a [128,1024] table tile
                ap = t[:]
                return bass.AP(ap.tensor, ap.offset + off,
                               [ap.ap[0], [512, 2], [1, 256]])

            def emit_tb(r):
                # B-table synthesis: negate half on DVE, copy half on ACT,
                # both h-halves per op (fewer instructions -> shorter
                # teardown sem chains)
                nc.vector.tensor_scalar_mul(_hp(TB[r], 0), _hp(TA[r], 256), -1.0)
                nc.scalar.copy(_hp(TB[r], 256), _hp(TA[r], 0))

            load_u(0)
            load_ta(0)
            load_ta(1)
            load_u(1)
            load_ta(2)
            load_ta(3)
            load_u(2)
            load_u(3)
            load_u(4)
            emit_tb(0)
            emit_tb(1)

            def emit_fft4(ci, b0_first=False):
                b, lo, hi = CHUNKS[ci]
                C = hi - lo
                U = UT[ci]

                def u(j):
                    return U[:, j * 2 * C:(j + 1) * 2 * C]

                Bt = {}
                pqw = {}
                tt = nc.vector.tensor_tensor

                def prereq(h):
                    u0, u1, u2, u3 = u(h), u(2 + h), u(4 + h), u(6 + h)
                    P = tmpp.tile([128, 2 * C], bf16, tag=f"P{h}", name=f"P{h}_{ci}")
                    Q = tmpp.tile([128, 2 * C], bf16, tag=f"Q{h}", name=f"Q{h}_{ci}")
                    U2 = tmpp.tile([128, 2 * C], bf16, tag=f"U{h}", name=f"U{h}_{ci}")
                    W = tmpp.tile([128, 2 * C], bf16, tag=f"W{h}", name=f"W{h}_{ci}")
                    for r in range(4):
                        Bt[(r, h)] = bbp.tile(
                            [128, 2 * C], bf16, tag=f"b{r}{h}", name=f"b{r}{h}_{ci}"
                        )
                    tt(P[:], u0, u2, op=ALU.add)
                    tt(Q[:], u0, u2, op=ALU.subtract)
                    tt(U2[:], u1, u3, op=ALU.add)
                    # W = (V.im | -V.re), V = u1 - u3; B1 = Q+W, B3 = Q-W
                    tt(W[:, 0:C], u1[:, C:2 * C], u3[:, C:2 * C], op=ALU.subtract)
                    tt(W[:, C:2 * C], u3[:, 0:C], u1[:, 0:C], op=ALU.subtract)
                    pqw[h] = (P, Q, U2, W)

                def emit_b(r, h):
                    P, Q, U2, W = pqw[h]
                    if r == 0:
                        tt(Bt[(0, h)][:], P[:], U2[:], op=ALU.add)
                    elif r == 1:
                        tt(Bt[(1, h)][:], Q[:], W[:], op=ALU.add)
                    elif r == 2:
                        tt(Bt[(2, h)][:], P[:], U2[:], op=ALU.subtract)
                    else:
                        tt(Bt[(3, h)][:], Q[:], W[:], op=ALU.subtract)

                if b0_first:
                    prereq(0)
                    for r in range(4):
                        emit_b(r, 0)
                    emit_tb(2)
                    prereq(1)
                    for r in range(4):
                        emit_b(r, 1)
                    emit_tb(3)
                else:
                    for h in range(2):
                        prereq(h)
                        for r in range(4):
                            emit_b(r, h)
                return Bt

            def emit_kblock(ci, Bt, kb, split_store=False, bb=0):
                # kb is the global kblock index (k rows 128*kb..128*kb+128)
                b, lo, hi = CHUNKS[ci]
                C = hi - lo
                c0 = 128 * kb - lo
                chi_t = chip.tile([128, N], bf16, tag="chi", name=f"chi{ci}_{kb}")
                sqm = None
                if not split_store:
                    # one [128,2048] sq tile per kblock -> single merged
                    # pair-add (fewer DVE ops: shorter runtime AND teardown)
                    sqm = sqp.tile([128, 2048], bf16, tag="sqm", name=f"sqm{ci}{kb}")
                for r in range(4):
                    ps = psp.tile([128, 512], f32, tag="ps", name=f"ps{ci}{kb}{r}")
                    if ci == 0:
                        # h-major MM order: h1's B-tiles land ~1us after h0's
                        # at the pipeline head, so do both h0 MMs first
                        order = [("A", 0), ("B", 0), ("A", 1), ("B", 1)]
                    else:
                        order = [("A", 0), ("A", 1), ("B", 0), ("B", 1)]
                    for i, (form, h) in enumerate(order):
                        off = c0 if form == "A" else C + c0
                        st = Bt[(r, h)][:, off:off + 128]
                        nc.tensor.matmul(ps[:], st, TT(form, r, h),
                                         start=(i == 0), stop=(i == 3))
                    if split_store:
                        # final kblock: per-r squares/pair-adds so each
                        # quarter drains via the idle SP queue immediately
                        sq = sqp.tile([128, 512], bf16, tag="sq", name=f"sq{ci}{kb}{r}")
                        nc.scalar.square(sq[:], ps[:])
                        nc.vector.tensor_tensor(
                            chi_t[:, r * 256:(r + 1) * 256],
                            sq[:, 0:256], sq[:, 256:512], op=ALU.add,
                        )
                        nc.sync.dma_start(
                            out[bb, 128 * kb:128 * kb + 128, r * 256:(r + 1) * 256],
                            chi_t[:, r * 256:(r + 1) * 256],
                        )
                    else:
                        nc.scalar.square(sqm[:, r * 512:(r + 1) * 512], ps[:])
                if not split_store:
                    # chi[r*256+q] = sq[r*512+q] + sq[r*512+256+q]
                    sap = sqm[:]
                    nc.vector.tensor_tensor(
                        chi_t[:],
                        bass.AP(sap.tensor, sap.offset,
                                [sap.ap[0], [512, 4], [1, 256]]),
                        bass.AP(sap.tensor, sap.offset + 256,
                                [sap.ap[0], [512, 4], [1, 256]]),
                        op=ALU.add,
                    )
                return chi_t

            def emit_store(b, kb, chi_t):
                # SP queue: keeps store descriptor-generation out of ACT's
                # FIFO and its sems off ACT's teardown chain
                nc.sync.dma_start(out[b, 128 * kb:128 * kb + 128, :], chi_t[:])

            # ---- schedule ----
            last_ci = len(CHUNKS) - 1
            for ci, (b, lo, hi) in enumerate(CHUNKS):
                Bt = emit_fft4(ci, b0_first=(ci == 0))
                for kb in range(lo // 128, hi // 128):
                    final = (ci == last_ci and kb == hi // 128 - 1)
                    chi_t = emit_kblock(ci, Bt, kb, split_store=final, bb=b)
                    if not final:
                        emit_store(b, kb, chi_t)

    _split_excess_waits(nc)
    return nc


_NC_CACHE = {}


def _get_nc():
    if "nc" not in _NC_CACHE:
        _NC_CACHE["nc"] = build_nc()
    return _NC_CACHE["nc"]


def _get_tables():
    if "tabs" not in _NC_CACHE:
        mpp = np.arange(256, dtype=np.float64)[:, None]
        t = np.arange(256, dtype=np.float64)[None, :]
        t_sh = (t + 128) % 256
        blocks = []
        for r in range(4):
            ang = 2.0 * np.pi * ((mpp * (r + 4 * t_sh)) % 1024) / 1024
            Mc = np.cos(ang)
            Ms = np.sin(ang)
            for h in range(2):
                sl = slice(128 * h, 128 * h + 128)
                blocks.append(np.concatenate([Mc[sl], -Ms[sl]], axis=1))
        big = np.concatenate(blocks, axis=1)
        _NC_CACHE["tabs"] = big.astype(ml_dtypes.bfloat16)
    return _NC_CACHE["tabs"]


def _host_prep(sr, si):
    """Per-core input prep. sr/si: [BPC, N] float32 (already prescaled).

    Chunk-major rt: for chunk ci=(b,lo,hi), block [128, 16*C] at CH_OFF[ci]
    where rt[p, off + j*2C + reim*C + c] = {re,im} R[k=lo+c, t=128j+p]."""
    Rts = []
    for b in range(BPC):
        s = sr[b].astype(np.complex64)
        s.imag = si[b]
        cs = np.conj(s)
        arr = np.concatenate([cs, cs])
        Wm = np.lib.stride_tricks.as_strided(
            arr[N:], shape=(N, K), strides=(arr.itemsize, -arr.itemsize))
        R = s[:, None] * Wm  # [t, k]
        Rb = np.empty((N, 2, K), dtype=np.float32)
        Rb[:, 0, :] = R.real
        Rb[:, 1, :] = R.imag
        Rts.append(Rb)
    rt = np.empty((128, 2 * 16 * K), dtype=ml_dtypes.bfloat16)
    for ci, (b, lo, hi) in enumerate(CHUNKS):
        C = hi - lo
        blk = Rts[b][:, :, lo:hi]                        # [t, 2, C]
        blk = blk.reshape(8, 128, 2, C).transpose(1, 0, 2, 3)  # [p, j, 2, C]
        rt[:, CH_OFF[ci]:CH_OFF[ci] + 16 * C] = blk.reshape(128, 16 * C).astype(
            ml_dtypes.bfloat16)
    return {"rt": rt, "tabs": _get_tables()}


def kernel(s_real: np.ndarray, s_imag: np.ndarray) -> np.ndarray:
    s_real = np.asarray(s_real, dtype=np.float32)
    s_imag = np.asarray(s_imag, dtype=np.float32)
    # exact normalization: max chi = (sum |s|^2)^2 (Cauchy-Schwarz, attained
    # at k=0,f=0), so prescale s by (sum|s|^2)^{-1/2}
    pw = (s_real.astype(np.float64) ** 2 + s_imag.astype(np.float64) ** 2).sum(
        axis=1, keepdims=True
    )
    g = 1.0 / np.sqrt(pw)
    sr_s = (s_real * g).astype(np.float32)
    si_s = (s_imag * g).astype(np.float32)

    nc = _get_nc()
    in_maps = [
        _host_prep(sr_s[c * BPC:(c + 1) * BPC], si_s[c * BPC:(c + 1) * BPC])
        for c in range(NCORES)
    ]
    res = bass_utils.run_bass_kernel_spmd(nc, in_maps, core_ids=list(range(NCORES)))
    planes = np.concatenate([r["out"] for r in res.results], axis=0)  # [B,512,N] bf16
    # r-plane interleave: chi[k, 4q+r] = planes[k, r*256+q]
    chi = (
        planes.astype(np.float32)
        .reshape(B, K, 4, 256)
        .transpose(0, 1, 3, 2)
        .reshape(B, K, N)
    )

    full = np.empty((B, N, N), dtype=np.float32)
    full[:, 512:1024, :] = chi
    # mirror: rows r in [1,512): chi[r] = flip_f(chi_direct[512 - r])
    src = chi[:, 511:0:-1, :]
    full[:, 1:512, 0] = src[:, :, 0]
    full[:, 1:512, 1:] = src[:, :, :0:-1]
    # row 0 (k=512) on host in float64
    s64 = (sr_s.astype(np.float64) + 1j * si_s.astype(np.float64))
    r512 = s64 * np.conj(np.roll(s64, 512, axis=1))
    x512 = np.fft.fft(r512, axis=1)
    full[:, 0, :] = np.fft.fftshift(
        (x512 * np.conj(x512)).real, axes=-1
    ).astype(np.float32)
    return full
